# revision 1
# baseline (speedup 1.0000x reference)
"""CorrCosine TRN2 kernel.

out[b, i, j, h, w] = <cur[b,:,i,j]/||cur[b,:,i,j]||, ref[b,:,h,w]/||ref[b,:,h,w]||>

Data-parallel over batch B=8 across the 8 NeuronCores; per core one
[4096 x 256] @ [256 x 4096] GEMM in fp32r (TF32) plus the two L2
normalizations, fused by pre-scaling both operands with 1/norm computed
on-chip (sum over C via an all-ones stationary matmul, which also leaves
the result broadcast across all 128 partitions).
"""

import numpy as np

from concourse import bacc, mybir
from concourse import tile
from concourse.bass_utils import run_bass_kernel_spmd

B, C, H, W = 8, 256, 64, 64
HW = H * W            # 4096
P = 128               # partitions
KT = C // P           # 2 k-tiles
FD = 512              # psum bank free dim (fp32)
NCH = HW // FD        # 8 column chunks
MT = HW // P          # 32 m-tiles
OBW = 4096            # output staging width (2 MiB DMAs)
IBW = 2048            # input DMA width (1 MiB chunks, lets norm start early)

f32 = mybir.dt.float32
f32r = mybir.dt.float32r
AF = mybir.ActivationFunctionType

_cached_nc = None


def _build():
    nc = bacc.Bacc("TRN2", target_bir_lowering=False, debug=False)
    cur_d = nc.dram_tensor("cur", [C, HW], f32, kind="ExternalInput")
    ref_d = nc.dram_tensor("ref", [C, HW], f32, kind="ExternalInput")
    out_d = nc.dram_tensor("out", [HW, HW], f32, kind="ExternalOutput")

    with tile.TileContext(nc) as tc:
        with (
            tc.tile_pool(name="scl", bufs=1) as sclp,
            tc.tile_pool(name="cst", bufs=1) as cstp,
            tc.tile_pool(name="ps", bufs=8, space="PSUM") as psp,
        ):
            ones_f = cstp.tile([P, P], f32, tag="ones_f", name="ones_f")
            nc.gpsimd.memset(ones_f[:], 1.0)
            ones = cstp.tile([P, P], f32r, tag="ones", name="ones")
            nc.vector.tensor_copy(ones[:], ones_f[:])

            # ref gets pre-scaled (column scaling); cur is consumed raw (f32r)
            # and its 1/norm is applied as a per-partition scale during PSUM
            # evacuation instead.
            scl = {}
            for k in range(KT):
                scl["ref", k] = sclp.tile([P, HW], f32r, tag=f"sref{k}", name=f"scl_ref{k}")
            cur_r = {}
            for k in range(KT):
                cur_r[k] = sclp.tile([P, HW], f32r, tag=f"curr{k}", name=f"cur_r{k}")
            # inv_cur in column layout: invcur_col[p, m] = 1/||cur[:, m*128+p]||
            invcur = cstp.tile([P, MT], f32, tag="invcur", name="invcur")

            # --- normalization: per 512-column chunk, both k-tiles ---
            # ref first so the main GEMM (which needs every ref chunk but
            # only one cur chunk per m-tile) can start as early as possible.
            with (
                tc.tile_pool(name="raw", bufs=1) as rawp,
                tc.tile_pool(name="sq", bufs=3) as sqp,
                tc.tile_pool(name="nrm", bufs=2) as nrmp,
            ):
                raw = {}
                for k in range(KT):
                    raw["ref", k] = rawp.tile(
                        [P, HW], f32, tag=f"ref{k}", name=f"raw_ref{k}"
                    )
                # halves-first order: ref h0 x2 -> cur h0 x2 -> ref h1 -> cur h1,
                # so ref-chunk normalization starts after just two 1 MiB DMAs.
                # cur is DMA-cast straight to f32r (SWDGE dtype cast).
                for i in range(HW // IBW):
                    for k in range(KT):
                        nc.gpsimd.dma_start(
                            raw["ref", k][:, i * IBW:(i + 1) * IBW],
                            ref_d[k * P:(k + 1) * P, i * IBW:(i + 1) * IBW],
                        )
                    for k in range(KT):
                        nc.gpsimd.dma_start(
                            cur_r[k][:, i * IBW:(i + 1) * IBW],
                            cur_d[k * P:(k + 1) * P, i * IBW:(i + 1) * IBW],
                        )

                def norm_ref_chunk(ch):
                    sl = slice(ch * FD, (ch + 1) * FD)
                    sq0 = sqp.tile([P, FD], f32r, tag="sq", name="sq0")
                    nc.scalar.activation(sq0[:], raw["ref", 0][:, sl], AF.Square)
                    sq1 = sqp.tile([P, FD], f32r, tag="sq", name="sq1")
                    nc.scalar.activation(sq1[:], raw["ref", 1][:, sl], AF.Square)
                    # sum over C: ones.T @ sq, broadcast on all partitions
                    ss = psp.tile([P, FD], f32, tag="ss", name="ss", bufs=2)
                    nc.tensor.matmul(ss[:], ones[:], sq0[:], start=True, stop=False)
                    nc.tensor.matmul(ss[:], ones[:], sq1[:], start=False, stop=True)
                    nrm = nrmp.tile([P, FD], f32, tag="nrm", name="nrm")
                    nc.scalar.activation(nrm[:], ss[:], AF.Sqrt)
                    inv = nrmp.tile([P, FD], f32, tag="inv", name="inv")
                    nc.vector.reciprocal_approx_fast(inv[:], nrm[:])
                    # scale-muls on the otherwise-idle GpSimd engine, keeping
                    # DVE/ACT free for the GEMM's PSUM evacuation copies
                    nc.gpsimd.tensor_mul(scl["ref", 0][:, sl], raw["ref", 0][:, sl], inv[:])
                    nc.gpsimd.tensor_mul(scl["ref", 1][:, sl], raw["ref", 1][:, sl], inv[:])

                def norm_cur_chunk(ch):
                    # squares of the cur chunk (plain f32), then per-m-tile
                    # column sums via fp32 matmul: sq stationary, ones vector
                    # moving -> psum [128, 4] column layout; sqrt + 1/x.
                    sl = slice(ch * FD, (ch + 1) * FD)
                    sq0 = sqp.tile([P, FD], f32, tag="sq", name="sq0")
                    nc.scalar.activation(sq0[:], cur_r[0][:, sl], AF.Square)
                    sq1 = sqp.tile([P, FD], f32, tag="sq", name="sq1")
                    nc.scalar.activation(sq1[:], cur_r[1][:, sl], AF.Square)
                    mpc = FD // P  # m-tiles per chunk (4)
                    pc = psp.tile([P, mpc], f32, tag="ss", name="pc", bufs=2)
                    for q in range(mpc):
                        qsl = slice(q * P, (q + 1) * P)
                        nc.tensor.matmul(
                            pc[:, q:q + 1], sq0[:, qsl], ones_f[:, 0:1],
                            start=True, stop=False,
                        )
                        nc.tensor.matmul(
                            pc[:, q:q + 1], sq1[:, qsl], ones_f[:, 0:1],
                            start=False, stop=True,
                        )
                    ncol = nrmp.tile([P, mpc], f32, tag="ncol", name="ncol")
                    nc.scalar.activation(ncol[:], pc[:], AF.Sqrt)
                    nc.vector.reciprocal_approx_fast(
                        invcur[:, ch * mpc:(ch + 1) * mpc], ncol[:]
                    )

                for ch in range(NCH):
                    norm_ref_chunk(ch)

                # --- main GEMM: out[m*128 :, :] = inv_cur[m] * cur.T @ ref_s ---
                # interleaved with cur normalization: chunk ch of cur feeds
                # m-tiles 4ch..4ch+3, so out-DMA starts after ~9/16 of norm.
                with tc.tile_pool(name="outp", bufs=3) as outp:
                    ndma = 0
                    for m in range(MT):
                        if m % (MT // NCH) == 0:
                            norm_cur_chunk(m // (MT // NCH))
                        msl = slice(m * P, (m + 1) * P)
                        mscale = invcur[:, m:m + 1]
                        for half in range(HW // OBW):
                            ob = outp.tile([P, OBW], f32, tag="ob", name="ob")
                            # 2-bank psum tiles: 4 matmuls in, one wide copy out
                            for q in range(OBW // (2 * FD)):
                                pt = psp.tile(
                                    [P, 2 * FD], f32, tag="pt", name="pt", bufs=3
                                )
                                for sub in range(2):
                                    n = half * (OBW // FD) + q * 2 + sub
                                    nsl = slice(n * FD, (n + 1) * FD)
                                    psl = slice(sub * FD, (sub + 1) * FD)
                                    nc.tensor.matmul(
                                        pt[:, psl], cur_r[0][:, msl],
                                        scl["ref", 0][:, nsl],
                                        start=True, stop=False,
                                    )
                                    nc.tensor.matmul(
                                        pt[:, psl], cur_r[1][:, msl],
                                        scl["ref", 1][:, nsl],
                                        start=False, stop=True,
                                    )
                                osl = slice(q * 2 * FD, (q + 1) * 2 * FD)
                                # evacuate with the cur row scale fused in,
                                # balanced between ACT and DVE
                                if q % 2 == 0:
                                    nc.scalar.activation(
                                        ob[:, osl], pt[:], AF.Copy, scale=mscale
                                    )
                                else:
                                    nc.vector.tensor_scalar_mul(
                                        ob[:, osl], pt[:], mscale
                                    )
                            # alternate the two HWDGE rings (SP / ACT)
                            eng = nc.sync if ndma % 2 == 0 else nc.scalar
                            ndma += 1
                            eng.dma_start(
                                out_d[msl, half * OBW:(half + 1) * OBW], ob[:]
                            )

    nc.compile()
    return nc


def _get_nc():
    global _cached_nc
    if _cached_nc is None:
        _cached_nc = _build()
    return _cached_nc


def _run(cur, ref, trace=False, **kw):
    """cur/ref: [B, C, HW] float32. Returns (out [B, HW, HW], results)."""
    nc = _get_nc()
    in_maps = [{"cur": cur[b], "ref": ref[b]} for b in range(B)]
    res = run_bass_kernel_spmd(nc, in_maps, list(range(B)), trace=trace, **kw)
    out = np.stack([res.results[b]["out"] for b in range(B)])
    return out, res


def kernel(ref_features, cur_features):
    ref = np.ascontiguousarray(np.asarray(ref_features, np.float32).reshape(B, C, HW))
    cur = np.ascontiguousarray(np.asarray(cur_features, np.float32).reshape(B, C, HW))
    out, _ = _run(cur, ref)
    return out.reshape(B, H, W, H, W)



# revision 4
# speedup vs baseline: 1.4273x; 1.4273x over previous
"""CorrCosine TRN2 kernel.

out[b, i, j, h, w] = <cur[b,:,i,j]/||cur[b,:,i,j]||, ref[b,:,h,w]/||ref[b,:,h,w]||>

Data-parallel over batch B=8 across the 8 NeuronCores; per core one
[4096 x 256] @ [256 x 4096] GEMM in bf16 plus the two L2 normalizations.

Both operands are pre-scaled with 1/norm (computed on-chip via an
all-ones stationary matmul that leaves the column sums broadcast across
all 128 partitions), so PSUM evacuation is a plain fp32->bf16 copy.
Inputs are host-cast to bf16 (halves input HBM traffic); the output is
written as bf16 and host-upcast to fp32 (halves the dominant output
traffic: 33.5 MB/core instead of 67 MB).

The main loop runs column-stripe-major (4 stripes of 1024 output
columns x 32 row-tiles): stripe s only needs ref block s normalized, so
the PE starts after the first 1024 ref columns are ready instead of
waiting for the full ref normalization.
"""

import numpy as np
import ml_dtypes

from concourse import bacc, mybir
from concourse import tile
from concourse.bass_utils import run_bass_kernel_spmd

B, C, H, W = 8, 256, 64, 64
HW = H * W            # 4096
P = 128               # partitions
KT = C // P           # 2 k-tiles
FD = 512              # psum bank free dim (fp32)
BW = 1024             # norm / stripe block width
NB = HW // BW         # 4 blocks
MT = HW // P          # 32 m-tiles

f32 = mybir.dt.float32
bf16 = mybir.dt.bfloat16
AF = mybir.ActivationFunctionType

_cached_nc = None


def _build():
    nc = bacc.Bacc("TRN2", target_bir_lowering=False, debug=False)
    cur_d = nc.dram_tensor("cur", [C, HW], bf16, kind="ExternalInput")
    ref_d = nc.dram_tensor("ref", [C, HW], bf16, kind="ExternalInput")
    out_d = nc.dram_tensor("out", [HW, HW], bf16, kind="ExternalOutput")

    with tile.TileContext(nc) as tc:
        with (
            tc.tile_pool(name="cst", bufs=1) as cstp,
            tc.tile_pool(name="dat", bufs=1) as datp,
            tc.tile_pool(name="sq", bufs=2) as sqp,
            tc.tile_pool(name="nrm", bufs=2) as nrmp,
            tc.tile_pool(name="ps", bufs=8, space="PSUM") as psp,
            tc.tile_pool(name="outp", bufs=6) as obp,
        ):
            ones = cstp.tile([P, P], bf16, tag="ones", name="ones")
            nc.gpsimd.memset(ones[:], 1.0)

            raw = {}
            scl = {}
            for t in ("ref", "cur"):
                for k in range(KT):
                    raw[t, k] = datp.tile(
                        [P, HW], bf16, tag=f"raw_{t}{k}", name=f"raw_{t}{k}"
                    )
                    scl[t, k] = datp.tile(
                        [P, HW], bf16, tag=f"scl_{t}{k}", name=f"scl_{t}{k}"
                    )
            inv = {
                "ref": datp.tile([P, HW], f32, tag="inv_ref", name="inv_ref"),
                "cur": datp.tile([P, HW], f32, tag="inv_cur", name="inv_cur"),
            }

            # --- input DMAs: ref k0 on the sync ring, ref k1 on the scalar
            # (ACT) ring, cur on gpsimd SWDGE — three queues in parallel,
            # ref prioritized since block 0 gates the PE start ---
            for b in range(NB):
                bsl = slice(b * BW, (b + 1) * BW)
                nc.sync.dma_start(raw["ref", 0][:, bsl], ref_d[0:P, bsl])
            for b in range(NB):
                bsl = slice(b * BW, (b + 1) * BW)
                nc.scalar.dma_start(raw["ref", 1][:, bsl], ref_d[P:2 * P, bsl])
            for b in range(NB):
                bsl = slice(b * BW, (b + 1) * BW)
                for k in range(KT):
                    nc.gpsimd.dma_start(
                        raw["cur", k][:, bsl], cur_d[k * P:(k + 1) * P, bsl]
                    )

            def sq_block(t, b, eng):
                """Squares of block b of tensor t. Returns (sq0, sq1)."""
                bsl = slice(b * BW, (b + 1) * BW)
                sq0 = sqp.tile([P, BW], bf16, tag=f"sq_{t}0", name=f"sq_{t}0")
                sq1 = sqp.tile([P, BW], bf16, tag=f"sq_{t}1", name=f"sq_{t}1")
                if eng == "act":
                    nc.scalar.activation(sq0[:], raw[t, 0][:, bsl], AF.Square)
                    nc.scalar.activation(sq1[:], raw[t, 1][:, bsl], AF.Square)
                elif eng == "dve":
                    nc.vector.tensor_mul(sq0[:], raw[t, 0][:, bsl], raw[t, 0][:, bsl])
                    nc.vector.tensor_mul(sq1[:], raw[t, 1][:, bsl], raw[t, 1][:, bsl])
                elif eng == "mix":
                    nc.scalar.activation(sq0[:], raw[t, 0][:, bsl], AF.Square)
                    nc.gpsimd.tensor_mul(sq1[:], raw[t, 1][:, bsl], raw[t, 1][:, bsl])
                else:
                    nc.gpsimd.tensor_mul(sq0[:], raw[t, 0][:, bsl], raw[t, 0][:, bsl])
                    nc.gpsimd.tensor_mul(sq1[:], raw[t, 1][:, bsl], raw[t, 1][:, bsl])
                return sq0, sq1

            def fin_block(t, b, sq0, sq1):
                """Sum (ones-matmul, broadcast to all partitions), sqrt,
                reciprocal, and pre-scale both k-tiles of block b."""
                bsl = slice(b * BW, (b + 1) * BW)
                nrm = nrmp.tile([P, BW], f32, tag="nrm", name="nrm")
                for c in range(BW // FD):
                    csl = slice(c * FD, (c + 1) * FD)
                    ss = psp.tile([P, FD], f32, tag="ss", name="ss", bufs=2)
                    nc.tensor.matmul(ss[:], ones[:], sq0[:, csl],
                                     start=True, stop=False)
                    nc.tensor.matmul(ss[:], ones[:], sq1[:, csl],
                                     start=False, stop=True)
                    nc.scalar.activation(nrm[:, csl], ss[:], AF.Sqrt)
                nc.vector.reciprocal_approx_fast(inv[t][:, bsl], nrm[:])
                for k in range(KT):
                    nc.vector.tensor_mul(
                        scl[t, k][:, bsl], raw[t, k][:, bsl], inv[t][:, bsl]
                    )

            # --- normalization preamble ---
            # Block 0 of both tensors avoids Pool entirely (the Pool queue
            # is busy issuing cur's SWDGE input DMAs); later blocks use
            # Pool for squares. DVE order: ref b0, cur b0, ref b1..b3
            # (cur b1-b3 finish inside the main loop).
            ref_sq0 = sq_block("ref", 0, "act")
            cur_sq0 = sq_block("cur", 0, "dve")
            fin_block("ref", 0, *ref_sq0)
            fin_block("cur", 0, *cur_sq0)
            cur_sqs = {0: None}
            for b in range(1, NB):
                s0, s1 = sq_block("ref", b, "mix")
                cur_sqs[b] = sq_block("cur", b, "pool")
                fin_block("ref", b, s0, s1)

            # --- main GEMM, stripe-major ---
            ei = 0
            for s in range(NB):
                ssl = slice(s * BW, (s + 1) * BW)
                for m in range(MT):
                    if s == 0 and m % 8 == 4 and m // 8 + 1 < NB:
                        b = m // 8 + 1
                        fin_block("cur", b, *cur_sqs[b])
                    msl = slice(m * P, (m + 1) * P)
                    pt = psp.tile([P, BW], f32, tag="pt", name="pt", bufs=3)
                    for k in range(KT):
                        for c in range(BW // FD):
                            nsl = slice(s * BW + c * FD, s * BW + (c + 1) * FD)
                            nc.tensor.matmul(
                                pt[:, c * FD:(c + 1) * FD],
                                scl["cur", k][:, msl],
                                scl["ref", k][:, nsl],
                                start=(k == 0), stop=(k == KT - 1),
                            )
                    ob = obp.tile([P, BW], bf16, tag="ob", name="ob")
                    if ei % 3 < 2:
                        nc.scalar.activation(ob[:], pt[:], AF.Copy)
                    else:
                        nc.vector.tensor_copy(ob[:], pt[:])
                    ei += 1
                    nc.sync.dma_start(out_d[msl, ssl], ob[:])

    nc.compile()
    return nc


def _get_nc():
    global _cached_nc
    if _cached_nc is None:
        _cached_nc = _build()
    return _cached_nc


def _run(cur, ref, trace=False, **kw):
    """cur/ref: [B, C, HW] float32 or bf16. Returns (out [B,HW,HW] f32, res)."""
    nc = _get_nc()
    cur = np.asarray(cur).astype(ml_dtypes.bfloat16)
    ref = np.asarray(ref).astype(ml_dtypes.bfloat16)
    in_maps = [{"cur": cur[b], "ref": ref[b]} for b in range(B)]
    res = run_bass_kernel_spmd(nc, in_maps, list(range(B)), trace=trace, **kw)
    out = np.stack(
        [res.results[b]["out"].astype(np.float32) for b in range(B)]
    )
    return out, res


def kernel(ref_features, cur_features):
    ref = np.ascontiguousarray(np.asarray(ref_features, np.float32).reshape(B, C, HW))
    cur = np.ascontiguousarray(np.asarray(cur_features, np.float32).reshape(B, C, HW))
    out, _ = _run(cur, ref)
    return out.reshape(B, H, W, H, W)


# revision 8
# speedup vs baseline: 1.5985x; 1.1199x over previous
"""CorrCosine TRN2 kernel.

out[b, i, j, h, w] = <cur[b,:,i,j]/||cur[b,:,i,j]||, ref[b,:,h,w]/||ref[b,:,h,w]||>

Data-parallel over batch B=8 across the 8 NeuronCores; per core one
[4096 x 256] @ [256 x 4096] GEMM in bf16 plus the two L2 normalizations.

Both operands are pre-scaled with 1/norm (computed on-chip via an
all-ones stationary matmul that leaves the column sums broadcast across
all 128 partitions), so PSUM evacuation is a plain fp32->bf16 copy.
Inputs are host-cast to bf16 (halves input HBM traffic); the output is
written as bf16 and host-upcast to fp32 (halves the dominant output
traffic: 33.5 MB/core instead of 67 MB).

The main loop runs column-stripe-major (4 stripes of 1024 output
columns x 32 row-tiles): stripe s only needs ref block s normalized, so
the PE starts after the first 1024 ref columns are ready instead of
waiting for the full ref normalization.
"""

import numpy as np
import ml_dtypes

from concourse import bacc, mybir
from concourse import tile
from concourse.bass_utils import run_bass_kernel_spmd

B, C, H, W = 8, 256, 64, 64
HW = H * W            # 4096
P = 128               # partitions
KT = C // P           # 2 k-tiles
FD = 512              # psum bank free dim (fp32)
BW = 1024             # norm / stripe block width
NB = HW // BW         # 4 blocks
MT = HW // P          # 32 m-tiles

f32 = mybir.dt.float32
bf16 = mybir.dt.bfloat16
AF = mybir.ActivationFunctionType

_cached_nc = None


def _build():
    nc = bacc.Bacc("TRN2", target_bir_lowering=False, debug=False)
    cur_d = nc.dram_tensor("cur", [C, HW], bf16, kind="ExternalInput")
    ref_d = nc.dram_tensor("ref", [C, HW], bf16, kind="ExternalInput")
    out_d = nc.dram_tensor("out", [HW, HW], bf16, kind="ExternalOutput")

    with tile.TileContext(nc) as tc:
        with (
            tc.tile_pool(name="cst", bufs=1) as cstp,
            tc.tile_pool(name="dat", bufs=1) as datp,
            tc.tile_pool(name="sq", bufs=2) as sqp,
            tc.tile_pool(name="nrm", bufs=2) as nrmp,
            tc.tile_pool(name="ps", bufs=8, space="PSUM") as psp,
            tc.tile_pool(name="outp", bufs=8) as obp,
        ):
            ones = cstp.tile([P, P], bf16, tag="ones", name="ones")
            nc.gpsimd.memset(ones[:], 1.0)

            raw = {}
            scl = {}
            for t in ("ref", "cur"):
                for k in range(KT):
                    raw[t, k] = datp.tile(
                        [P, HW], bf16, tag=f"raw_{t}{k}", name=f"raw_{t}{k}"
                    )
                    scl[t, k] = datp.tile(
                        [P, HW], bf16, tag=f"scl_{t}{k}", name=f"scl_{t}{k}"
                    )
            inv = {
                "ref": datp.tile([P, HW], f32, tag="inv_ref", name="inv_ref"),
                "cur": datp.tile([P, HW], f32, tag="inv_cur", name="inv_cur"),
            }

            # --- input DMAs: all of ref on the sync ring (block 0 first,
            # both k-tiles), cur on gpsimd SWDGE — keeps the ACT queue free
            # for the startup-critical squares ---
            for b in range(NB):
                bsl = slice(b * BW, (b + 1) * BW)
                for k in range(KT):
                    nc.sync.dma_start(
                        raw["ref", k][:, bsl], ref_d[k * P:(k + 1) * P, bsl]
                    )
            for b in range(NB):
                bsl = slice(b * BW, (b + 1) * BW)
                for k in range(KT):
                    nc.gpsimd.dma_start(
                        raw["cur", k][:, bsl], cur_d[k * P:(k + 1) * P, bsl]
                    )

            def act_sq(t, b):
                """Squares of 1024-block b of tensor t on ACT."""
                bsl = slice(b * BW, (b + 1) * BW)
                sq0 = sqp.tile([P, BW], bf16, tag=f"sq_{t}0", name=f"sq_{t}0")
                sq1 = sqp.tile([P, BW], bf16, tag=f"sq_{t}1", name=f"sq_{t}1")
                nc.scalar.activation(sq0[:], raw[t, 0][:, bsl], AF.Square)
                nc.scalar.activation(sq1[:], raw[t, 1][:, bsl], AF.Square)
                return sq0, sq1

            def sums_sqrt(t, b, sq0, sq1, nrm):
                """Ones-matmul column sums (broadcast) + sqrt for block b."""
                for c in range(BW // FD):
                    csl = slice(c * FD, (c + 1) * FD)
                    ss = psp.tile([P, FD], f32, tag="ss", name="ss", bufs=2)
                    nc.tensor.matmul(ss[:], ones[:], sq0[:, csl],
                                     start=True, stop=False)
                    nc.tensor.matmul(ss[:], ones[:], sq1[:, csl],
                                     start=False, stop=True)
                    nc.scalar.activation(nrm[:, csl], ss[:], AF.Sqrt)

            # --- block-0 normalization (gates the PE start) ---
            # ACT: all four squares + sqrts; DVE: both recips then ref
            # scales at 512 granularity; Pool: cur scales (its SWDGE
            # issues are done by then). Chunk-0 scales first so the
            # (k0, chunk0) matmul of m-tile 0 can start ASAP.
            rsq0, rsq1 = act_sq("ref", 0)
            csq0, csq1 = act_sq("cur", 0)
            rnrm = nrmp.tile([P, BW], f32, tag="nrm", name="nrm")
            cnrm = nrmp.tile([P, BW], f32, tag="nrm", name="nrm")
            sums_sqrt("ref", 0, rsq0, rsq1, rnrm)
            sums_sqrt("cur", 0, csq0, csq1, cnrm)
            nc.vector.reciprocal_approx_fast(inv["ref"][:, 0:BW], rnrm[:])
            nc.vector.reciprocal_approx_fast(inv["cur"][:, 0:BW], cnrm[:])
            for c in range(BW // FD):
                csl = slice(c * FD, (c + 1) * FD)
                for k in range(KT):
                    nc.vector.tensor_mul(
                        scl["ref", k][:, csl], raw["ref", k][:, csl],
                        inv["ref"][:, csl],
                    )
                    nc.gpsimd.tensor_mul(
                        scl["cur", k][:, csl], raw["cur", k][:, csl],
                        inv["cur"][:, csl],
                    )

            # ref blocks 1-3 are normalized lazily, spread across the
            # stripes — ref block s is only needed when stripe s begins,
            # and the full chain (squares..scales, ~9 us) fits in the half
            # stripe between m-tile 16 and the next stripe start.
            def ref_fin(b):
                bsl = slice(b * BW, (b + 1) * BW)
                sq0, sq1 = act_sq("ref", b)
                nrm = nrmp.tile([P, BW], f32, tag="nrm", name="nrm")
                sums_sqrt("ref", b, sq0, sq1, nrm)
                nc.vector.reciprocal_approx_fast(inv["ref"][:, bsl], nrm[:])
                for k in range(KT):
                    nc.vector.tensor_mul(
                        scl["ref", k][:, bsl], raw["ref", k][:, bsl],
                        inv["ref"][:, bsl],
                    )

            def cur_chunk_fin(c):
                """Normalize cur chunk c (512 cols): ACT squares+sqrt, DVE
                recip + k0 scale, Pool k1 scale."""
                csl = slice(c * FD, (c + 1) * FD)
                sq0 = sqp.tile([P, FD], bf16, tag="csq0", name="csq0")
                sq1 = sqp.tile([P, FD], bf16, tag="csq1", name="csq1")
                nc.scalar.activation(sq0[:], raw["cur", 0][:, csl], AF.Square)
                nc.scalar.activation(sq1[:], raw["cur", 1][:, csl], AF.Square)
                ss = psp.tile([P, FD], f32, tag="ss", name="ss", bufs=2)
                nc.tensor.matmul(ss[:], ones[:], sq0[:], start=True, stop=False)
                nc.tensor.matmul(ss[:], ones[:], sq1[:], start=False, stop=True)
                nrm = nrmp.tile([P, FD], f32, tag="nrmc", name="nrmc")
                nc.scalar.activation(nrm[:], ss[:], AF.Sqrt)
                nc.vector.reciprocal_approx_fast(inv["cur"][:, csl], nrm[:])
                nc.vector.tensor_mul(
                    scl["cur", 0][:, csl], raw["cur", 0][:, csl],
                    inv["cur"][:, csl],
                )
                nc.gpsimd.tensor_mul(
                    scl["cur", 1][:, csl], raw["cur", 1][:, csl],
                    inv["cur"][:, csl],
                )

            # --- main GEMM, stripe-major ---
            ei = 0
            for s in range(NB):
                ssl = slice(s * BW, (s + 1) * BW)
                for m in range(MT):
                    if s == 0 and m % 4 == 0 and m // 4 + 2 < 2 * NB:
                        cur_chunk_fin(m // 4 + 2)
                    if m == 16 and 0 < s + 1 < NB:
                        ref_fin(s + 1)
                    msl = slice(m * P, (m + 1) * P)
                    pt = psp.tile([P, BW], f32, tag="pt", name="pt", bufs=3)
                    for k in range(KT):
                        for c in range(BW // FD):
                            nsl = slice(s * BW + c * FD, s * BW + (c + 1) * FD)
                            nc.tensor.matmul(
                                pt[:, c * FD:(c + 1) * FD],
                                scl["cur", k][:, msl],
                                scl["ref", k][:, nsl],
                                start=(k == 0), stop=(k == KT - 1),
                            )
                    ob = obp.tile([P, BW], bf16, tag="ob", name="ob")
                    # stripe 0: 1:1 evac split (ACT also runs cur norms);
                    # later stripes: 2/3 on ACT.
                    on_act = (ei % 2 == 0) if s == 0 else (ei % 3 < 2)
                    if on_act:
                        nc.scalar.activation(ob[:], pt[:], AF.Copy)
                    else:
                        nc.vector.tensor_copy(ob[:], pt[:])
                    ei += 1
                    nc.sync.dma_start(out_d[msl, ssl], ob[:])

    nc.compile()
    return nc


def _get_nc():
    global _cached_nc
    if _cached_nc is None:
        _cached_nc = _build()
    return _cached_nc


def _run(cur, ref, trace=False, **kw):
    """cur/ref: [B, C, HW] float32 or bf16. Returns (out [B,HW,HW] f32, res)."""
    nc = _get_nc()
    cur = np.asarray(cur).astype(ml_dtypes.bfloat16)
    ref = np.asarray(ref).astype(ml_dtypes.bfloat16)
    in_maps = [{"cur": cur[b], "ref": ref[b]} for b in range(B)]
    res = run_bass_kernel_spmd(nc, in_maps, list(range(B)), trace=trace, **kw)
    out = np.stack(
        [res.results[b]["out"].astype(np.float32) for b in range(B)]
    )
    return out, res


def kernel(ref_features, cur_features):
    ref = np.ascontiguousarray(np.asarray(ref_features, np.float32).reshape(B, C, HW))
    cur = np.ascontiguousarray(np.asarray(cur_features, np.float32).reshape(B, C, HW))
    out, _ = _run(cur, ref)
    return out.reshape(B, H, W, H, W)


# revision 9
# speedup vs baseline: 1.6074x; 1.0056x over previous
"""CorrCosine TRN2 kernel.

out[b, i, j, h, w] = <cur[b,:,i,j]/||cur[b,:,i,j]||, ref[b,:,h,w]/||ref[b,:,h,w]||>

Data-parallel over batch B=8 across the 8 NeuronCores; per core one
[4096 x 256] @ [256 x 4096] GEMM in bf16 plus the two L2 normalizations.

Both operands are pre-scaled with 1/norm (computed on-chip via an
all-ones stationary matmul that leaves the column sums broadcast across
all 128 partitions), so PSUM evacuation is a plain fp32->bf16 copy.
Inputs are host-cast to bf16 (halves input HBM traffic); the output is
written as bf16 and host-upcast to fp32 (halves the dominant output
traffic: 33.5 MB/core instead of 67 MB).

The main loop runs column-stripe-major (4 stripes of 1024 output
columns x 32 row-tiles): stripe s only needs ref block s normalized, so
the PE starts after the first 1024 ref columns are ready instead of
waiting for the full ref normalization.
"""

import numpy as np
import ml_dtypes

from concourse import bacc, mybir
from concourse import tile
from concourse.bass_utils import run_bass_kernel_spmd

B, C, H, W = 8, 256, 64, 64
HW = H * W            # 4096
P = 128               # partitions
KT = C // P           # 2 k-tiles
FD = 512              # psum bank free dim (fp32)
BW = 1024             # norm / stripe block width
NB = HW // BW         # 4 blocks
MT = HW // P          # 32 m-tiles

f32 = mybir.dt.float32
bf16 = mybir.dt.bfloat16
AF = mybir.ActivationFunctionType

_cached_nc = None


def _build():
    nc = bacc.Bacc("TRN2", target_bir_lowering=False, debug=False)
    cur_d = nc.dram_tensor("cur", [C, HW], bf16, kind="ExternalInput")
    ref_d = nc.dram_tensor("ref", [C, HW], bf16, kind="ExternalInput")
    out_d = nc.dram_tensor("out", [HW, HW], bf16, kind="ExternalOutput")

    with tile.TileContext(nc) as tc:
        with (
            tc.tile_pool(name="cst", bufs=1) as cstp,
            tc.tile_pool(name="dat", bufs=1) as datp,
            tc.tile_pool(name="sq", bufs=2) as sqp,
            tc.tile_pool(name="nrm", bufs=2) as nrmp,
            tc.tile_pool(name="ps", bufs=8, space="PSUM") as psp,
            tc.tile_pool(name="outp", bufs=8) as obp,
        ):
            ones = cstp.tile([P, P], bf16, tag="ones", name="ones")
            nc.gpsimd.memset(ones[:], 1.0)

            raw = {}
            scl = {}
            for t in ("ref", "cur"):
                for k in range(KT):
                    raw[t, k] = datp.tile(
                        [P, HW], bf16, tag=f"raw_{t}{k}", name=f"raw_{t}{k}"
                    )
                    scl[t, k] = datp.tile(
                        [P, HW], bf16, tag=f"scl_{t}{k}", name=f"scl_{t}{k}"
                    )
            inv = {
                "ref": datp.tile([P, HW], f32, tag="inv_ref", name="inv_ref"),
                "cur": datp.tile([P, HW], f32, tag="inv_cur", name="inv_cur"),
            }

            # --- input DMAs: all of ref on the sync ring (block 0 first,
            # both k-tiles), cur on gpsimd SWDGE — keeps the ACT queue free
            # for the startup-critical squares ---
            for b in range(NB):
                bsl = slice(b * BW, (b + 1) * BW)
                for k in range(KT):
                    nc.sync.dma_start(
                        raw["ref", k][:, bsl], ref_d[k * P:(k + 1) * P, bsl]
                    )
            for b in range(NB):
                bsl = slice(b * BW, (b + 1) * BW)
                for k in range(KT):
                    nc.gpsimd.dma_start(
                        raw["cur", k][:, bsl], cur_d[k * P:(k + 1) * P, bsl]
                    )

            def act_sq(t, b):
                """Squares of 1024-block b of tensor t on ACT."""
                bsl = slice(b * BW, (b + 1) * BW)
                sq0 = sqp.tile([P, BW], bf16, tag=f"sq_{t}0", name=f"sq_{t}0")
                sq1 = sqp.tile([P, BW], bf16, tag=f"sq_{t}1", name=f"sq_{t}1")
                nc.scalar.activation(sq0[:], raw[t, 0][:, bsl], AF.Square)
                nc.scalar.activation(sq1[:], raw[t, 1][:, bsl], AF.Square)
                return sq0, sq1

            def sums_sqrt(t, b, sq0, sq1, nrm):
                """Ones-matmul column sums (broadcast) + sqrt for block b."""
                for c in range(BW // FD):
                    csl = slice(c * FD, (c + 1) * FD)
                    ss = psp.tile([P, FD], f32, tag="ss", name="ss", bufs=2)
                    nc.tensor.matmul(ss[:], ones[:], sq0[:, csl],
                                     start=True, stop=False)
                    nc.tensor.matmul(ss[:], ones[:], sq1[:, csl],
                                     start=False, stop=True)
                    nc.scalar.activation(nrm[:, csl], ss[:], AF.Sqrt)

            # --- block-0 normalization (gates the PE start) ---
            # ACT: all four squares + sqrts; DVE: both recips then ref
            # scales at 512 granularity; Pool: cur scales (its SWDGE
            # issues are done by then). Chunk-0 scales first so the
            # (k0, chunk0) matmul of m-tile 0 can start ASAP.
            rsq0, rsq1 = act_sq("ref", 0)
            csq0, csq1 = act_sq("cur", 0)
            rnrm = nrmp.tile([P, BW], f32, tag="nrm", name="nrm")
            cnrm = nrmp.tile([P, BW], f32, tag="nrm", name="nrm")
            sums_sqrt("ref", 0, rsq0, rsq1, rnrm)
            sums_sqrt("cur", 0, csq0, csq1, cnrm)
            nc.vector.reciprocal_approx_fast(inv["ref"][:, 0:BW], rnrm[:])
            nc.vector.reciprocal_approx_fast(inv["cur"][:, 0:BW], cnrm[:])
            for c in range(BW // FD):
                csl = slice(c * FD, (c + 1) * FD)
                for k in range(KT):
                    nc.vector.tensor_mul(
                        scl["ref", k][:, csl], raw["ref", k][:, csl],
                        inv["ref"][:, csl],
                    )
                    nc.gpsimd.tensor_mul(
                        scl["cur", k][:, csl], raw["cur", k][:, csl],
                        inv["cur"][:, csl],
                    )

            # ref blocks 1-3 are normalized lazily, spread across the
            # stripes — ref block s is only needed when stripe s begins,
            # and the full chain (squares..scales, ~9 us) fits in the half
            # stripe between m-tile 16 and the next stripe start.
            def ref_fin(b):
                bsl = slice(b * BW, (b + 1) * BW)
                sq0, sq1 = act_sq("ref", b)
                nrm = nrmp.tile([P, BW], f32, tag="nrm", name="nrm")
                sums_sqrt("ref", b, sq0, sq1, nrm)
                nc.vector.reciprocal_approx_fast(inv["ref"][:, bsl], nrm[:])
                for k in range(KT):
                    nc.vector.tensor_mul(
                        scl["ref", k][:, bsl], raw["ref", k][:, bsl],
                        inv["ref"][:, bsl],
                    )

            def cur_chunk_fin(c):
                """Normalize cur chunk c (512 cols): ACT squares+sqrt, DVE
                recip + k0 scale, Pool k1 scale."""
                csl = slice(c * FD, (c + 1) * FD)
                sq0 = sqp.tile([P, FD], bf16, tag="csq0", name="csq0")
                sq1 = sqp.tile([P, FD], bf16, tag="csq1", name="csq1")
                nc.scalar.activation(sq0[:], raw["cur", 0][:, csl], AF.Square)
                nc.scalar.activation(sq1[:], raw["cur", 1][:, csl], AF.Square)
                ss = psp.tile([P, FD], f32, tag="ss", name="ss", bufs=2)
                nc.tensor.matmul(ss[:], ones[:], sq0[:], start=True, stop=False)
                nc.tensor.matmul(ss[:], ones[:], sq1[:], start=False, stop=True)
                nrm = nrmp.tile([P, FD], f32, tag="nrmc", name="nrmc")
                nc.scalar.activation(nrm[:], ss[:], AF.Sqrt)
                nc.vector.reciprocal_approx_fast(inv["cur"][:, csl], nrm[:])
                nc.vector.tensor_mul(
                    scl["cur", 0][:, csl], raw["cur", 0][:, csl],
                    inv["cur"][:, csl],
                )
                nc.gpsimd.tensor_mul(
                    scl["cur", 1][:, csl], raw["cur", 1][:, csl],
                    inv["cur"][:, csl],
                )

            # cur chunks 2-3 normalized before the main loop (their squares
            # only depend on the input DMA); chunks 4-7 are issued three
            # m-tile groups ahead of first use inside stripe 0.
            cur_chunk_fin(2)
            cur_chunk_fin(3)

            # --- main GEMM, stripe-major ---
            ei = 0
            for s in range(NB):
                ssl = slice(s * BW, (s + 1) * BW)
                for m in range(MT):
                    if s == 0 and m % 4 == 0 and m // 4 + 4 < 2 * NB:
                        cur_chunk_fin(m // 4 + 4)
                    if m == 16 and 0 < s + 1 < NB:
                        ref_fin(s + 1)
                    msl = slice(m * P, (m + 1) * P)
                    pt = psp.tile([P, BW], f32, tag="pt", name="pt", bufs=3)
                    for k in range(KT):
                        for c in range(BW // FD):
                            nsl = slice(s * BW + c * FD, s * BW + (c + 1) * FD)
                            nc.tensor.matmul(
                                pt[:, c * FD:(c + 1) * FD],
                                scl["cur", k][:, msl],
                                scl["ref", k][:, nsl],
                                start=(k == 0), stop=(k == KT - 1),
                            )
                    ob = obp.tile([P, BW], bf16, tag="ob", name="ob")
                    # stripe 0: 1:1 evac split (ACT also runs cur norms);
                    # later stripes: 2/3 on ACT.
                    on_act = (ei % 2 == 0) if s == 0 else (ei % 3 < 2)
                    if on_act:
                        nc.scalar.activation(ob[:], pt[:], AF.Copy)
                    else:
                        nc.vector.tensor_copy(ob[:], pt[:])
                    ei += 1
                    nc.sync.dma_start(out_d[msl, ssl], ob[:])

    nc.compile()
    return nc


def _get_nc():
    global _cached_nc
    if _cached_nc is None:
        _cached_nc = _build()
    return _cached_nc


def _run(cur, ref, trace=False, **kw):
    """cur/ref: [B, C, HW] float32 or bf16. Returns (out [B,HW,HW] f32, res)."""
    nc = _get_nc()
    cur = np.asarray(cur).astype(ml_dtypes.bfloat16)
    ref = np.asarray(ref).astype(ml_dtypes.bfloat16)
    in_maps = [{"cur": cur[b], "ref": ref[b]} for b in range(B)]
    res = run_bass_kernel_spmd(nc, in_maps, list(range(B)), trace=trace, **kw)
    out = np.stack(
        [res.results[b]["out"].astype(np.float32) for b in range(B)]
    )
    return out, res


def kernel(ref_features, cur_features):
    ref = np.ascontiguousarray(np.asarray(ref_features, np.float32).reshape(B, C, HW))
    cur = np.ascontiguousarray(np.asarray(cur_features, np.float32).reshape(B, C, HW))
    out, _ = _run(cur, ref)
    return out.reshape(B, H, W, H, W)
